# revision 10
# baseline (speedup 1.0000x reference)
"""BioMech feature extraction on Trainium2: 8 NeuronCores, pure data-parallel SPMD.

Self-contained: takes full inputs foot/shank/thigh [8192, 12, 256] fp32,
returns [8192, 44] fp32 feature matrix matching the reference stack order.

Strategy per core (1024 samples, 8 blocks of 128 partitions):
  - Host packs the 22 used channels -> X [B, 22, 256] (foot z+gyro, shank
    z+gyro, thigh gyro), one DMA per 128-sample block.
  - FFT features via PE matmul with DFT weight matrices (cos|sin and
    sqrt(k)-scaled variants), after a PE transpose of the z-channels.
  - Sums/variances via DVE bn_stats; peaks via tensor_scalar abs_max with
    max-accumulator; central moments via fused tensor_tensor_reduce;
    ACT Square/Abs with sum-accumulators.
  - Per-sample scalar math batched over all blocks at the end.
"""

import numpy as np

import concourse.bacc as bacc
import concourse.tile as tile
import concourse.mybir as mybir
from concourse.bass_utils import run_bass_kernel_spmd

F32 = mybir.dt.float32
AF = mybir.ActivationFunctionType
ALU = mybir.AluOpType
AX = mybir.AxisListType

N_CORES = 8
B_FULL = 8192
T = 256
P = 128
BC = B_FULL // N_CORES          # 1024 samples per core
NBLK = BC // P                  # 8 blocks
NCH = 22
NBINS = 129
HF_BIN = 60
EPS = 1e-6

CH_FOOT = [2, 3, 4, 5, 8, 9, 10, 11]
CH_SHANK = [2, 3, 4, 5, 8, 9, 10, 11]
CH_THIGH = [3, 4, 5, 9, 10, 11]

FZ = (0, 4)          # packed idx of foot z lt/rt
SZ = (8, 12)         # packed idx of shank z lt/rt
GROUPS = (1, 5, 9, 13, 16, 19)   # gyro triples: fgL fgR sgL sgR tgL tgR


def build_consts():
    t = np.arange(T, dtype=np.float64)
    k = np.arange(NBINS, dtype=np.float64)
    ang = 2.0 * np.pi * np.outer(t, k) / T
    C = np.cos(ang)
    S = np.sin(ang)
    wa = np.concatenate([C, S], axis=1)                    # [256, 258]
    sk = np.sqrt(k)
    wb = np.concatenate([C * sk, S * sk], axis=1)          # [256, 258]
    wa = np.ascontiguousarray(wa.reshape(2, P, 2 * NBINS), dtype=np.float32)
    wb = np.ascontiguousarray(wb.reshape(2, P, 2 * NBINS), dtype=np.float32)
    wh = np.zeros((2, P, 2), dtype=np.float32)
    wh[0, :, 0] = 1.0   # chunk 0 (t<128) -> first-half abs sum
    wh[1, :, 1] = 1.0   # chunk 1 (t>=128) -> second-half abs sum
    ident = np.eye(P, dtype=np.float32)
    return {"wa": wa, "wb": wb, "wh": wh, "ident": ident}


def build_nc():
    nc = bacc.Bacc("TRN2", target_bir_lowering=False, debug=False,
                   num_devices=N_CORES)
    x_d = nc.dram_tensor("x", [BC, NCH, T], F32, kind="ExternalInput")
    wa_d = nc.dram_tensor("wa", [2, P, 2 * NBINS], F32, kind="ExternalInput")
    wb_d = nc.dram_tensor("wb", [2, P, 2 * NBINS], F32, kind="ExternalInput")
    wh_d = nc.dram_tensor("wh", [2, P, 2], F32, kind="ExternalInput")
    id_d = nc.dram_tensor("ident", [P, P], F32, kind="ExternalInput")
    out_d = nc.dram_tensor("out", [BC, 44], F32, kind="ExternalOutput")

    with tile.TileContext(nc) as tc:
        _body(tc, x_d, wa_d, wb_d, wh_d, id_d, out_d)
    nc.compile()
    return nc


def _body(tc, x_d, wa_d, wb_d, wh_d, id_d, out_d):
    nc = tc.nc
    NS = NBLK * 2        # number of (block, side) stat slots
    NG = NBLK * 6        # number of (block, group) slots

    import contextlib
    ctx = contextlib.ExitStack()
    with ctx:
        pers = ctx.enter_context(tc.tile_pool(name="pers", bufs=1))
        p_in = ctx.enter_context(tc.tile_pool(name="xin", bufs=3))
        p_psxt = ctx.enter_context(tc.tile_pool(name="psxt", bufs=2, space="PSUM"))
        p_psmm = ctx.enter_context(tc.tile_pool(name="psmm", bufs=2, space="PSUM"))
        p_psh = ctx.enter_context(tc.tile_pool(name="psh", bufs=2, space="PSUM"))
        p_xt = ctx.enter_context(tc.tile_pool(name="xt", bufs=2))
        p_p2 = ctx.enter_context(tc.tile_pool(name="p2", bufs=2))
        p_scr = ctx.enter_context(tc.tile_pool(name="scr", bufs=3))
        p_junk = ctx.enter_context(tc.tile_pool(name="junk", bufs=6))
        p_small = ctx.enter_context(tc.tile_pool(name="small", bufs=3))
        fin = ctx.enter_context(tc.tile_pool(name="fin", bufs=1))

        # ---- constants to SBUF ----
        wa_sb = pers.tile([P, 2, 2 * NBINS], F32, tag="wa")
        wb_sb = pers.tile([P, 2, 2 * NBINS], F32, tag="wb")
        wh_sb = pers.tile([P, 2, 2], F32, tag="wh")
        id_sb = pers.tile([P, P], F32, tag="ident")
        nc.sync.dma_start(out=wa_sb[:], in_=wa_d.ap().rearrange("j p n -> p j n"))
        nc.sync.dma_start(out=wb_sb[:], in_=wb_d.ap().rearrange("j p n -> p j n"))
        nc.sync.dma_start(out=wh_sb[:], in_=wh_d.ap().rearrange("j p n -> p j n"))
        nc.sync.dma_start(out=id_sb[:], in_=id_d.ap())

        # ---- persistent per-(block,side) stats [P, NS] ----
        def stat(tag, n=NS):
            return pers.tile([P, n], F32, tag=tag, name=tag)

        pkF_s = stat("pkF")     # max|fz|
        pkS_s = stat("pkS")     # max|sz|
        tot_s = stat("tot")     # sum power (all 129 bins)
        scn_s = stat("scn")     # sum k*power
        hf_s = stat("hf")       # sum power k>=60
        mu_s = stat("mu")       # mean fz
        m2_s = stat("m2")       # sum (fz-mu)^2
        m3_s = stat("m3")       # sum (fz-mu)^3
        m4_s = stat("m4")       # sum (fz-mu)^4
        imp_s = stat("imp")     # count |fz| >= 0.3 pk
        zc_s = stat("zc")       # zero crossing count
        vib_s = stat("vib")     # sum |diff sz|
        habs_s = stat("habs", 2 * NS)          # [b, side, half] flat
        bnS_s = pers.tile([P, NS, 6], F32, tag="bnS")
        bnG_s = pers.tile([P, NG * 3, 6], F32, tag="bnG")
        pkG_s = stat("pkG", NG)
        out_t = pers.tile([P, NBLK, 44], F32, tag="out")

        x_ap = x_d.ap()

        for b in range(NBLK):
            Xb = p_in.tile([P, NCH, T], F32, tag="xb")
            nc.sync.dma_start(out=Xb[:], in_=x_ap[b * P:(b + 1) * P, :, :])

            # ---------- foot z channels (FFT + moments + impact + zcr) ----------
            for side, ch in enumerate(FZ):
                x = Xb[:, ch, :]
                i2 = 2 * b + side
                xt_ps = p_psxt.tile([P, 2, P], F32, tag="xtps")
                nc.tensor.transpose(xt_ps[:, 0, :], x[:, 0:P], id_sb[:])
                nc.tensor.transpose(xt_ps[:, 1, :], x[:, P:T], id_sb[:])
                XT = p_xt.tile([P, 2, P], F32, tag="xt")
                nc.scalar.copy(XT[:], xt_ps[:])
                AXT = p_xt.tile([P, 2, P], F32, tag="axt")
                nc.scalar.activation(AXT[:], xt_ps[:], AF.Abs)

                psA = p_psmm.tile([P, 2 * NBINS], F32, tag="psA")
                psB = p_psmm.tile([P, 2 * NBINS], F32, tag="psB")
                psH = p_psh.tile([P, 2], F32, tag="psH")
                for j in (0, 1):
                    nc.tensor.matmul(psA[:], XT[:, j, :], wa_sb[:, j, :],
                                     start=(j == 0), stop=(j == 1))
                for j in (0, 1):
                    nc.tensor.matmul(psB[:], XT[:, j, :], wb_sb[:, j, :],
                                     start=(j == 0), stop=(j == 1))
                for j in (0, 1):
                    nc.tensor.matmul(psH[:], AXT[:, j, :], wh_sb[:, j, :],
                                     start=(j == 0), stop=(j == 1))

                P2 = p_p2.tile([P, 2 * NBINS], F32, tag="p2")
                nc.scalar.activation(P2[:], psA[:], AF.Square,
                                     accum_out=tot_s[:, i2:i2 + 1])
                jB = p_junk.tile([P, 2 * NBINS], F32, tag="junk")
                nc.scalar.activation(jB[:], psB[:], AF.Square,
                                     accum_out=scn_s[:, i2:i2 + 1])
                hfv = P2[:].rearrange("p (h k) -> p h k", h=2)[:, :, HF_BIN:NBINS]
                nc.vector.reduce_sum(hf_s[:, i2:i2 + 1], hfv, axis=AX.XY)
                nc.scalar.copy(habs_s[:, 2 * i2:2 * i2 + 2], psH[:])
                nc.scalar.activation(mu_s[:, i2:i2 + 1], psA[:, 0:1], AF.Copy,
                                     scale=1.0 / T)

                ABSX = p_scr.tile([P, T], F32, tag="absx")
                nc.scalar.activation(ABSX[:], x, AF.Abs)
                nc.vector.tensor_reduce(pkF_s[:, i2:i2 + 1], x, axis=AX.X,
                                        op=ALU.max, apply_absolute_value=True)
                thr = p_small.tile([P, 1], F32, tag="thr")
                nc.vector.tensor_scalar(thr[:], pkF_s[:, i2:i2 + 1], 0.3, None,
                                        op0=ALU.mult)
                j1 = p_junk.tile([P, 2 * NBINS], F32, tag="junk")
                nc.vector.tensor_scalar(j1[:, 0:T], ABSX[:], thr[:], None,
                                        op0=ALU.is_ge, op1=ALU.add,
                                        accum_out=imp_s[:, i2:i2 + 1])

                C = p_scr.tile([P, T], F32, tag="c")
                nc.vector.tensor_scalar(C[:], x, mu_s[:, i2:i2 + 1], None,
                                        op0=ALU.subtract)
                C2 = p_scr.tile([P, T], F32, tag="c2")
                nc.scalar.activation(C2[:], C[:], AF.Square,
                                     accum_out=m2_s[:, i2:i2 + 1])
                j3 = p_junk.tile([P, 2 * NBINS], F32, tag="junk")
                nc.scalar.activation(j3[:, 0:T], C2[:], AF.Square,
                                     accum_out=m4_s[:, i2:i2 + 1])
                C3 = p_scr.tile([P, T], F32, tag="c3")
                nc.vector.tensor_mul(C3[:], C2[:], C[:])
                j2 = p_junk.tile([P, 2 * NBINS], F32, tag="junk")
                nc.scalar.activation(j2[:, 0:T], C3[:], AF.Copy,
                                     accum_out=m3_s[:, i2:i2 + 1])

                PR = p_scr.tile([P, T - 1], F32, tag="pr")
                nc.vector.tensor_mul(PR[:], x[:, 1:T], x[:, 0:T - 1])
                j4 = p_junk.tile([P, 2 * NBINS], F32, tag="junk")
                nc.vector.tensor_scalar(j4[:, 0:T - 1], PR[:], 0.0, None,
                                        op0=ALU.is_lt, op1=ALU.add,
                                        accum_out=zc_s[:, i2:i2 + 1])

            # ---------- shank z channels ----------
            for side, ch in enumerate(SZ):
                y = Xb[:, ch, :]
                i2 = 2 * b + side
                nc.vector.tensor_reduce(pkS_s[:, i2:i2 + 1], y, axis=AX.X,
                                        op=ALU.max, apply_absolute_value=True)
                nc.vector.bn_stats(bnS_s[:, i2, :], y)
                D = p_scr.tile([P, T - 1], F32, tag="d")
                nc.vector.tensor_sub(D[:], y[:, 1:T], y[:, 0:T - 1])
                j6 = p_junk.tile([P, 2 * NBINS], F32, tag="junk")
                nc.scalar.activation(j6[:, 0:T - 1], D[:], AF.Abs,
                                     accum_out=vib_s[:, i2:i2 + 1])

            # ---------- gyro groups ----------
            for gi, c0 in enumerate(GROUPS):
                G = Xb[:, c0:c0 + 3, :]
                ig = 6 * b + gi
                nc.vector.tensor_reduce(pkG_s[:, ig:ig + 1], G, axis=AX.XY,
                                        op=ALU.max, apply_absolute_value=True)
                for cc in range(3):
                    nc.vector.bn_stats(bnG_s[:, ig * 3 + cc, :], G[:, cc, :])

        # ================= final batched scalar phase =================
        def v2(tbl):
            return tbl[:].rearrange("p (b s) -> p b s", s=2)

        def ft(tag, shape=(P, NBLK, 2)):
            return fin.tile(list(shape), F32, tag=tag, name=tag)

        V = nc.vector
        SC = nc.scalar

        # direct peak copies
        V.tensor_copy(out_t[:, :, 0:2], v2(pkF_s))
        V.tensor_copy(out_t[:, :, 2:4], v2(pkS_s))

        # ratio = log1p(f_pk / (s_pk + 1e-4))  [Ln later]
        r_spk = ft("r_spk")
        V.tensor_scalar(r_spk[:], v2(pkS_s), 1e-4, None, op0=ALU.add)
        V.reciprocal(r_spk[:], r_spk[:])
        ratio_arg = ft("ratio_arg")
        V.tensor_mul(ratio_arg[:], v2(pkF_s), r_spk[:])

        # total power reciprocal (shared by hf and sc)
        r_tot = ft("r_tot")
        V.tensor_scalar(r_tot[:], v2(tot_s), EPS, None, op0=ALU.add)
        V.reciprocal(r_tot[:], r_tot[:])
        V.tensor_mul(out_t[:, :, 6:8], v2(hf_s), r_tot[:])
        V.scalar_tensor_tensor(out_t[:, :, 16:18], v2(scn_s), 1.0 / NBINS,
                               r_tot[:], op0=ALU.mult, op1=ALU.mult)

        # dur, vib, zcr scaled counts
        V.tensor_scalar(out_t[:, :, 18:20], v2(imp_s), 1.0 / T, None, op0=ALU.mult)
        V.tensor_scalar(out_t[:, :, 12:14], v2(vib_s), 1.0 / (T - 1), None, op0=ALU.mult)
        V.tensor_scalar(out_t[:, :, 42:44], v2(zc_s), 1.0 / (T - 1), None, op0=ALU.mult)

        # decay = h0 / (h1 + 128e-6)
        hv = habs_s[:].rearrange("p (b s h) -> p b s h", s=2, h=2)
        dden = ft("dden")
        V.tensor_scalar(dden[:], hv[:, :, :, 1], (T // 2) * EPS, None, op0=ALU.add)
        V.reciprocal(dden[:], dden[:])
        V.tensor_mul(out_t[:, :, 10:12], hv[:, :, :, 0], dden[:])

        # asym_acc = |f_pk - s_pk|
        aa = ft("aa")
        V.tensor_sub(aa[:], v2(pkF_s), v2(pkS_s))
        SC.activation(out_t[:, :, 32:34], aa[:], AF.Abs)

        # shank bn-derived: sum y^2 and M2 (=sum (y-mu)^2)
        bnSv = bnS_s[:].rearrange("p i s -> p i s")
        meS = bnS_s[:, :, 1].rearrange("p (b s) -> p b s", s=2)
        moS = bnS_s[:, :, 4].rearrange("p (b s) -> p b s", s=2)
        ceS = bnS_s[:, :, 2].rearrange("p (b s) -> p b s", s=2)
        coS = bnS_s[:, :, 5].rearrange("p (b s) -> p b s", s=2)
        sAS = ft("sAS")
        V.tensor_add(sAS[:], ceS, coS)
        uS = ft("uS")
        V.tensor_mul(uS[:], meS, meS)
        vS = ft("vS")
        V.tensor_mul(vS[:], moS, moS)
        wS = ft("wS")
        V.tensor_add(wS[:], uS[:], vS[:])
        sqS = ft("sqS")          # sum y^2
        V.scalar_tensor_tensor(sqS[:], wS[:], float(T // 2), sAS[:],
                               op0=ALU.mult, op1=ALU.add)
        musS = ft("musS")
        V.tensor_add(musS[:], meS, moS)
        qS = ft("qS")
        V.tensor_mul(qS[:], musS[:], musS[:])
        M2S = ft("M2S")          # sum (y - mu)^2
        V.scalar_tensor_tensor(M2S[:], qS[:], -float(T) / 4.0, sqS[:],
                               op0=ALU.mult, op1=ALU.add)

        # var_ratio = log1p(m2F / (M2S + 255e-4))  [Ln later]
        vr = ft("vr")
        V.tensor_scalar(vr[:], M2S[:], (T - 1) * 1e-4, None, op0=ALU.add)
        V.reciprocal(vr[:], vr[:])
        vra = ft("vra")
        V.tensor_mul(vra[:], v2(m2_s), vr[:])

        # foot sum x^2 = m2 + 256 mu^2
        qF = ft("qF")
        V.tensor_mul(qF[:], v2(mu_s), v2(mu_s))
        sqF = ft("sqF")
        V.scalar_tensor_tensor(sqF[:], qF[:], float(T), v2(m2_s),
                               op0=ALU.mult, op1=ALU.add)

        # gyro bn-derived per channel -> M2 summed over the 3 channels
        meG = bnG_s[:, :, 1]     # [P, NG*3] strided
        moG = bnG_s[:, :, 4]
        ceG = bnG_s[:, :, 2]
        coG = bnG_s[:, :, 5]
        NG3 = NG * 3
        sAG = ft("sAG", (P, NG3))
        V.tensor_add(sAG[:], ceG, coG)
        uG = ft("uG", (P, NG3))
        V.tensor_mul(uG[:], meG, meG)
        vG = ft("vG", (P, NG3))
        V.tensor_mul(vG[:], moG, moG)
        wG = ft("wG", (P, NG3))
        V.tensor_add(wG[:], uG[:], vG[:])
        sqG = ft("sqG", (P, NG3))
        V.scalar_tensor_tensor(sqG[:], wG[:], float(T // 2), sAG[:],
                               op0=ALU.mult, op1=ALU.add)
        musG = ft("musG", (P, NG3))
        V.tensor_add(musG[:], meG, moG)
        qG = ft("qG", (P, NG3))
        V.tensor_mul(qG[:], musG[:], musG[:])
        M2G = ft("M2G", (P, NG3))
        V.scalar_tensor_tensor(M2G[:], qG[:], -float(T) / 4.0, sqG[:],
                               op0=ALU.mult, op1=ALU.add)
        gM2 = ft("gM2", (P, NG))
        V.reduce_sum(gM2[:], M2G[:].rearrange("p (g c) -> p g c", c=3), axis=AX.X)

        # gyro peaks -> out columns 22,23 / 26,27 / 30,31
        vq = out_t[:, :, 20:32].rearrange("p b (j q) -> p b j q", q=4)
        V.tensor_copy(vq[:, :, :, 2:4],
                      pkG_s[:].rearrange("p (b j l) -> p b j l", j=3, l=2))

        # ---- Sqrt-set ACT ops ----
        SC.activation(out_t[:, :, 8:10], v2(m2_s), AF.Sqrt, scale=1.0 / (T - 1))
        rmsF = ft("rmsF")
        SC.activation(rmsF[:], sqF[:], AF.Sqrt, scale=1.0 / T)
        rmsS = ft("rmsS")
        SC.activation(rmsS[:], sqS[:], AF.Sqrt, scale=1.0 / T)

        # kurt/skew (uses std at out[...,8:10])
        sg = ft("sg")
        V.tensor_scalar(sg[:], out_t[:, :, 8:10], 1e-6, None, op0=ALU.max)
        vv = ft("vv")
        V.tensor_mul(vv[:], sg[:], sg[:])
        v4 = ft("v4")
        V.tensor_mul(v4[:], vv[:], vv[:])
        V.reciprocal(v4[:], v4[:])
        kr = ft("kr")
        V.scalar_tensor_tensor(kr[:], v2(m4_s), 1.0 / T, v4[:],
                               op0=ALU.mult, op1=ALU.mult)
        V.tensor_scalar(out_t[:, :, 38:40], kr[:], 30.0, -10.0,
                        op0=ALU.min, op1=ALU.max)
        v3 = ft("v3")
        V.tensor_mul(v3[:], vv[:], sg[:])
        V.reciprocal(v3[:], v3[:])
        sk = ft("sk")
        V.scalar_tensor_tensor(sk[:], v2(m3_s), 1.0 / T, v3[:],
                               op0=ALU.mult, op1=ALU.mult)
        V.tensor_scalar(out_t[:, :, 40:42], sk[:], 10.0, -10.0,
                        op0=ALU.min, op1=ALU.max)

        # trans arg = rmsS / (rmsF + 1e-6)
        rdn = ft("rdn")
        V.tensor_scalar(rdn[:], rmsF[:], EPS, None, op0=ALU.add)
        V.reciprocal(rdn[:], rdn[:])
        targ = ft("targ")
        V.tensor_mul(targ[:], rmsS[:], rdn[:])

        # ---- Ln-set ACT ops (log1p via bias=1) ----
        SC.activation(out_t[:, :, 4:6], ratio_arg[:], AF.Ln, bias=1.0)
        SC.activation(out_t[:, :, 14:16], vra[:], AF.Ln, bias=1.0)
        SC.activation(out_t[:, :, 36:38], targ[:], AF.Ln, bias=1.0)
        SC.activation(vq[:, :, :, 0:2],
                      gM2[:].rearrange("p (b j l) -> p b j l", j=3, l=2),
                      AF.Ln, scale=1.0 / (T - 1), bias=1.0)

        # asym_gy = |fg_var - sg_var| (after log1p)
        ag = ft("ag")
        V.tensor_sub(ag[:], out_t[:, :, 20:22], out_t[:, :, 24:26])
        SC.activation(out_t[:, :, 34:36], ag[:], AF.Abs)

        # ---- store ----
        nc.sync.dma_start(out=out_d.ap().rearrange("(b p) f -> p b f", p=P),
                          in_=out_t[:])


_NC_CACHE = None
_CONSTS = None


def _get_nc():
    global _NC_CACHE, _CONSTS
    if _NC_CACHE is None:
        _NC_CACHE = build_nc()
    if _CONSTS is None:
        _CONSTS = build_consts()
    return _NC_CACHE, _CONSTS


def run(foot, shank, thigh, **kw):
    foot = np.asarray(foot, dtype=np.float32)
    shank = np.asarray(shank, dtype=np.float32)
    thigh = np.asarray(thigh, dtype=np.float32)
    X = np.empty((B_FULL, NCH, T), dtype=np.float32)
    X[:, 0:8] = foot[:, CH_FOOT]
    X[:, 8:16] = shank[:, CH_SHANK]
    X[:, 16:22] = thigh[:, CH_THIGH]

    nc, consts = _get_nc()
    in_maps = []
    for i in range(N_CORES):
        in_maps.append({
            "x": np.ascontiguousarray(X[i * BC:(i + 1) * BC]),
            "wa": consts["wa"], "wb": consts["wb"], "wh": consts["wh"],
            "ident": consts["ident"],
        })
    return run_bass_kernel_spmd(nc, in_maps, core_ids=list(range(N_CORES)),
                                **kw)


def kernel(foot, shank, thigh):
    res = run(foot, shank, thigh)
    return np.concatenate([res.results[i]["out"] for i in range(N_CORES)],
                          axis=0)
